# revision 1
# baseline (speedup 1.0000x reference)
"""KAN expert kernel for Trainium2 (8 NeuronCores, data-parallel over batch).

Math: out[b,j] = sum_{i,g} basis_g(x[b,i]) * coeff[i,j,g] * scaling[i,j]
with cubic B-spline basis on the uniform extended grid g_m = -1 + 0.4*m.

Key identity (truncated powers): for the uniform grid, the basis is the
cardinal cubic B-spline, basis_g(x) = (1/(6h^3)) * sum_{r=0..4} w_r *
relu(x - g_{g+r})^3 with w = [1,-4,6,-4,1]. Since x in [-1,1) only
relu-features m=0..4 are nonzero, and the (linear) binomial combine is
folded into the weights on the host:
    C'[m,i,j] = (1/(6h^3)) * sum_g w_{m-g} * coeff[i,j,g] * scaling[i,j]
so each core computes Q_m = relu(x - g_m)^3 (m=0..4) and a single
[512b x 2560k] @ [2560k x 512j] fp32 matmul accumulated in PSUM.
"""

import numpy as np

BATCH = 4096
IN_DIM = 512
OUT_DIM = 512
GRID_SIZE = 5
K = 3
N_CORES = 8
P = 128
NM = 5                      # relu^3 feature channels
BC = BATCH // N_CORES       # 512 batch rows per core
NIC = IN_DIM // P           # 4 input-dim chunks

_W_BINOM = np.array([1.0, -4.0, 6.0, -4.0, 1.0])

_cached = {}


def _grid_f32():
    h = 2.0 / GRID_SIZE
    return np.float32(-1.0 + h * np.arange(GRID_SIZE + 2 * K + 1))


def _build_nc(mm_dtype_name="float32"):
    import concourse.bass as bass
    import concourse.mybir as mybir
    from concourse.tile import TileContext

    dt = mybir.dt
    mm_dt = getattr(dt, mm_dtype_name)
    grid = _grid_f32()

    nc = bass.Bass()
    xt = nc.dram_tensor("xt", [IN_DIM, BC], dt.float32, kind="ExternalInput")
    cw = nc.dram_tensor("cw", [NM * IN_DIM, OUT_DIM], dt.float32,
                        kind="ExternalInput")
    out = nc.dram_tensor("out", [BC, OUT_DIM], dt.float32,
                         kind="ExternalOutput")

    with TileContext(nc) as tc:
        with tc.tile_pool(name="main", bufs=1) as pool, \
             tc.tile_pool(name="psum", bufs=1, space="PSUM") as psum_pool:
            # x, transposed: partition = input-dim (128 of 512), free = (ic, b)
            X = pool.tile([P, NIC * BC], dt.float32, tag="X")
            nc.gpsimd.dma_start(
                out=X[:].rearrange("p (c b) -> p c b", c=NIC),
                in_=xt.rearrange("(c p) b -> p c b", p=P))

            # weight tiles: cw[(m*IN_DIM + ic*P) : +P, :]
            from concourse.bass import _add_dep_helper

            Wt = {}
            for m in range(NM):
                for ic in range(NIC):
                    w = pool.tile([P, OUT_DIM], mm_dt, tag=f"W{m}_{ic}")
                    r0 = m * IN_DIM + ic * P
                    nc.gpsimd.dma_start(out=w[:], in_=cw[r0:r0 + P, :])
                    Wt[(m, ic)] = w

            # features Q_m = relu(x - g_m)^3, computed across all (ic, b).
            # 1-elem ACT "probes" of each W tile precede the m-th square in
            # ACT program order (nosync deps), so a wait on the q-mult's DVE
            # tick transitively guarantees group-m weights have landed —
            # keeps every matmul at a single sync wait.
            scratch = pool.tile([1, 1], dt.float32, tag="scratch")
            Q = []
            for m in range(NM):
                gm = float(grid[m])
                probes = []
                for ic in range(NIC):
                    pr = nc.scalar.activation(
                        scratch[0:1, 0:1], Wt[(m, ic)][0:1, 0:1],
                        mybir.ActivationFunctionType.Copy)
                    probes.append(pr)
                r = pool.tile([P, NIC * BC], mm_dt, tag=f"r{m}")
                s = pool.tile([P, NIC * BC], dt.float32, tag=f"s{m}")
                # r = max(x - g_m, 0)            (DVE)
                nc.vector.tensor_scalar(
                    r[:], X[:], gm, 0.0,
                    mybir.AluOpType.subtract, mybir.AluOpType.max)
                # s = r^2                        (ACT)
                sq = nc.scalar.activation(
                    s[:], r[:], mybir.ActivationFunctionType.Square)
                for pr in probes:
                    _add_dep_helper(sq.ins, pr.ins, sync=False,
                                    reason="W-group probe before square")
                # q = r * s  (in-place into r)   (DVE)
                nc.vector.tensor_mul(r[:], r[:], s[:])
                Q.append(r)

            # matmuls: out[bc] = sum_{m,ic} Q_m[ic,bc-slice].T @ W[m,ic]
            psums = [psum_pool.tile([P, OUT_DIM], dt.float32, tag=f"ps{b}",
                                    name=f"ps{b}")
                     for b in range(BC // P)]
            n_k = NM * NIC
            for m in range(NM):
                for bc in range(BC // P):
                    for ic in range(NIC):
                        kc = m * NIC + ic
                        lhsT = Q[m][:, ic * BC + bc * P: ic * BC + (bc + 1) * P]
                        nc.tensor.matmul(
                            psums[bc][:], lhsT, Wt[(m, ic)][:],
                            start=(kc == 0), stop=(kc == n_k - 1))

            # evict psum -> sbuf (ACT) -> DRAM
            O = pool.tile([P, (BC // P) * OUT_DIM], dt.float32, tag="O")
            for bc in range(BC // P):
                nc.scalar.activation(
                    O[:, bc * OUT_DIM:(bc + 1) * OUT_DIM], psums[bc][:],
                    mybir.ActivationFunctionType.Copy)
            od = nc.gpsimd.dma_start(
                out=out.rearrange("(c p) j -> p c j", p=P),
                in_=O[:].rearrange("p (c j) -> p c j", c=BC // P))
            out_dmas = [od]

    # Walrus rejects >1 sync wait per compute instruction on this
    # toolchain. Two classes of waits are provably redundant here:
    #  - same-engine waits (DVE/ACT strict in-order FIFO; PE matmuls
    #    issue in order),
    #  - matmul DMASW waits (weight arrival is guaranteed through the
    #    probe -> square -> q-mult chain the matmul already waits on).
    eng2sem = {"EngineType.DVE": "DVE_",
               "EngineType.Activation": "Activation_",
               "EngineType.PE": "PE_"}
    bad = []
    for blk in nc.m.functions[0].blocks:
        for inst in blk.instructions:
            si = inst.sync_info
            if si is None or not si.on_wait:
                continue
            pref = eng2sem.get(str(inst.engine))
            keep = [w for w in si.on_wait
                    if pref is None
                    or not (w.ant_name or "").startswith(pref)]
            if type(inst).__name__ == "InstMatmult":
                dve = [w for w in keep
                       if (w.ant_name or "").startswith("DVE_")]
                if dve:
                    keep = dve
            if type(inst).__name__ == "InstDMACopy":
                # The only DMASW waits here are same-queue WAR ordering,
                # which the in-order SWDGE ring already guarantees.
                nq = [w for w in keep
                      if not (w.ant_name or "").startswith("DMASW")]
                if nq:
                    keep = nq
            if type(inst).__name__ == "InstDrain" and len(keep) > 8:
                # Keep only the out-DMA queue sems: everything else
                # (PE, ACT, DVE, input queues) is upstream of the
                # out-DMAs' wait chains.
                out_sems = {f"DMASW{od.ins.bass_scheduled_proc - 11}_"
                            for od in out_dmas}
                keep = [w for w in keep
                        if any((w.ant_name or "").startswith(s)
                               for s in out_sems)]
            if len(keep) != len(si.on_wait):
                si.on_wait = keep
            if len(keep) > 1 and type(inst).__name__ != "InstDrain":
                bad.append((inst.name, type(inst).__name__,
                            [w.ant_name for w in keep]))
    assert not bad, f"multi-wait compute instructions remain: {bad}"
    return nc


def _prep_weights(spline_coeff, spline_scaling):
    # C'[m,i,j] = (1/(6h^3)) * sum_g w[m-g] * coeff[i,j,g] * scaling[i,j]
    h = 2.0 / GRID_SIZE
    c = (spline_coeff.astype(np.float64)
         * spline_scaling.astype(np.float64)[:, :, None])  # [i, j, g]
    cp = np.zeros((NM, IN_DIM, OUT_DIM), np.float64)
    for m in range(NM):
        for g in range(max(0, m - 4), m + 1):
            cp[m] += _W_BINOM[m - g] * c[:, :, g]
    cp *= 1.0 / (6.0 * h ** 3)
    return np.ascontiguousarray(
        cp.reshape(NM * IN_DIM, OUT_DIM).astype(np.float32))


def _run(inputs, trace=False, mm_dtype_name="float32"):
    from concourse.bass_utils import run_bass_kernel_spmd

    key = mm_dtype_name
    if key not in _cached:
        _cached[key] = _build_nc(mm_dtype_name)
    nc = _cached[key]

    x = np.asarray(inputs["x"], np.float32)
    cw = _prep_weights(np.asarray(inputs["spline_coeff"]),
                       np.asarray(inputs["spline_scaling"]))
    in_maps = []
    for c in range(N_CORES):
        xc = np.ascontiguousarray(x[c * BC:(c + 1) * BC, :].T)
        in_maps.append({"xt": xc, "cw": cw})
    res = run_bass_kernel_spmd(nc, in_maps, list(range(N_CORES)),
                               trace=trace)
    outp = np.concatenate([res.results[c]["out"] for c in range(N_CORES)],
                          axis=0)
    return outp, res


def kernel(**inputs):
    outp, _ = _run(inputs, trace=False)
    return outp

